# revision 7
# baseline (speedup 1.0000x reference)
"""Conditional (class-routed) 3x3 SAME conv, data-parallel over batch on 8 TRN2 cores.

Strategy:
  - Host: gather per-sample expert kernel/bias (kernel[classes], bias[classes]),
    zero-pad x to 66x66 and transpose to channel-major [CIN, HP, WP]; shard
    batch 4 samples/core.
  - Device (per core): for each sample and output-channel half, process the
    64x64 output as 8 spatial chunks of 512 positions, CHUNK-MAJOR: the 9
    conv taps accumulate back-to-back into one PSUM bank ([CIN=128,
    FH=128]^T @ [CIN=128, 512] fp16 matmuls, fp32 PSUM), then the chunk is
    evicted (bias fused) on alternating Vector/Scalar engines while later
    chunks keep the PE busy.
  - x is staged twice in SBUF: plane 0 as-is (kw=0/2 taps) and plane 1
    shifted left one column (kw=1 taps) so every matmul rhs is 4-byte
    aligned (a 2-byte-misaligned rhs costs ~3% per matmul).
  - The first chunk's inputs (k taps 0:3, x rows 0:10) are triggered in
    parallel on the two HWDGE queues (Sync + Scalar) so the PE starts
    ~1us after the first DMA.  The very last chunk accumulates into two
    half-banks so its eviction runs Vector+Scalar in parallel.
  - Host: un-transpose [F, HW] -> [H, W, F] and concatenate shards.
"""

import numpy as np

_B, _H, _W, _CIN = 32, 64, 64, 128
_F, _KH, _KW = 256, 3, 3
_NCORES = 8
_BPC = _B // _NCORES          # 4 samples per core
_HP, _WP = _H + 2, _W + 2     # 66, 66 (zero-padded)
_SP = _H * _W                 # 4096 output positions
_FH = 128                     # output-channel half (PSUM partition dim)
_NFH = _F // _FH              # 2
_CHUNK = 512                  # spatial positions per PSUM bank
_NCH = _SP // _CHUNK          # 8
_ROWS = _CHUNK // _W          # 8 output rows per chunk
_NTAP = _KH * _KW             # 9

_nc = None


def _build_nc():
    import concourse.bacc as bacc
    import concourse.mybir as mybir
    import concourse.tile as tile
    from concourse.tile_rust import add_dep_helper

    f32 = mybir.dt.float32
    f16 = mybir.dt.float16
    ident = mybir.ActivationFunctionType.Identity

    nc = bacc.Bacc("TRN2", target_bir_lowering=False, debug=False)
    xT = nc.dram_tensor("xT", (_BPC, _CIN, _HP, _WP), f16, kind="ExternalInput")
    # kT[s, fh] is one f-half of the expert kernel: [CIN, NTAP, FH]
    kT = nc.dram_tensor("kT", (_BPC, _NFH, _CIN, _NTAP, _FH), f16,
                        kind="ExternalInput")
    bT = nc.dram_tensor("bT", (_FH, _BPC * _NFH), f32, kind="ExternalInput")
    yT = nc.dram_tensor("yT", (_BPC, _NFH, _FH, _SP), f16, kind="ExternalOutput")

    def rhs_ap(x_sb, c, tap, nrows=_ROWS, roff=0):
        kh, kw = divmod(tap, _KW)
        r0 = c * _ROWS + kh + roff
        if kw == 1:
            return x_sb[:, 1, r0:r0 + nrows, 0:_W]
        return x_sb[:, 0, r0:r0 + nrows, kw:kw + _W]

    with tile.TileContext(nc) as tc:
        with (
            tc.tile_pool(name="xp", bufs=3) as xp,
            tc.tile_pool(name="kp", bufs=5) as kp,
            tc.tile_pool(name="bp", bufs=1) as bp,
            tc.tile_pool(name="op", bufs=6) as op,
            tc.tile_pool(name="ps", bufs=8, space="PSUM") as ps,
        ):
            b_sb = bp.tile([_FH, _BPC * _NFH], f32)

            def load_x(eng, x_sb, s, r0, r1, plane):
                if plane == 0:
                    return eng.dma_start(x_sb[:, 0, r0:r1, :],
                                         xT[s, :, r0:r1, :])
                return eng.dma_start(x_sb[:, 1, r0:r1, 0:_WP - 1],
                                     xT[s, :, r0:r1, 1:_WP])

            gate_prev = None
            for s in range(_BPC):
                k_sb = []
                dmas = []
                # x plane 0: as-is; plane 1: shifted left one column
                x_sb = xp.tile([_CIN, 2, _HP, _WP], f16, name=f"x_s{s}",
                               tag="x")
                for fh in range(_NFH):
                    k_sb.append(kp.tile([_CIN, _NTAP, _FH], f16,
                                        name=f"k_s{s}f{fh}", tag="k"))
                if s == 0:
                    # interleave the two HWDGE trigger queues so the first
                    # chunk's inputs land ~1.3us after the first trigger
                    dmas.append(nc.sync.dma_start(k_sb[0][:, 0:3, :],
                                                  kT[s, 0, :, 0:3, :]))
                    dmas.append(load_x(nc.scalar, x_sb, s, 0, 10, 0))
                    dmas.append(load_x(nc.sync, x_sb, s, 0, 10, 1))
                    dmas.append(load_x(nc.scalar, x_sb, s, 10, 18, 0))
                    dmas.append(load_x(nc.sync, x_sb, s, 10, 18, 1))
                    dmas.append(nc.scalar.dma_start(k_sb[0][:, 3:_NTAP, :],
                                                    kT[s, 0, :, 3:_NTAP, :]))
                    dmas.append(load_x(nc.sync, x_sb, s, 18, 42, 0))
                    dmas.append(load_x(nc.scalar, x_sb, s, 18, 42, 1))
                    dmas.append(load_x(nc.sync, x_sb, s, 42, _HP, 0))
                    dmas.append(load_x(nc.scalar, x_sb, s, 42, _HP, 1))
                    dmas.append(nc.sync.dma_start(k_sb[1][:], kT[s, 1]))
                    nc.scalar.dma_start(b_sb[:], bT[:])
                else:
                    dmas.append(nc.sync.dma_start(k_sb[0][:], kT[s, 0]))
                    dmas.append(load_x(nc.scalar, x_sb, s, 0, 34, 0))
                    dmas.append(load_x(nc.sync, x_sb, s, 0, 34, 1))
                    dmas.append(load_x(nc.scalar, x_sb, s, 34, _HP, 0))
                    dmas.append(load_x(nc.sync, x_sb, s, 34, _HP, 1))
                    dmas.append(nc.scalar.dma_start(k_sb[1][:], kT[s, 1]))
                if gate_prev is not None:
                    # prefetch of sample s must not compete for HBM bandwidth
                    # with sample s-1's (still critical) input transfers
                    for d in dmas:
                        add_dep_helper(d.ins, gate_prev,
                                       reason="prefetch gated on prev sample")

                gate_this = None
                for fh in range(_NFH):
                    col = s * _NFH + fh
                    bias_ap = b_sb[:, col:col + 1]
                    o_sb = None
                    for c in range(_NCH):
                        split_last = (s == _BPC - 1 and fh == _NFH - 1
                                      and c == _NCH - 1)
                        if split_last:
                            # two half-banks so both evict engines can run
                            # in parallel on different banks at the very end
                            ph = [ps.tile([_FH, _CHUNK // 2], f32,
                                          name=f"psum_s{s}f{fh}c{c}h{h}",
                                          tag="psum") for h in range(2)]
                            for tap in range(_NTAP):
                                for h in range(2):
                                    nc.tensor.matmul(
                                        ph[h][:],
                                        k_sb[fh][:, tap, :],
                                        rhs_ap(x_sb, c, tap,
                                               nrows=_ROWS // 2,
                                               roff=h * _ROWS // 2),
                                        start=(tap == 0),
                                        stop=(tap == _NTAP - 1),
                                    )
                            half = _CHUNK // 2
                            nc.vector.tensor_scalar_add(
                                o_sb[:, _CHUNK:_CHUNK + half], ph[0][:],
                                bias_ap)
                            nc.scalar.activation(
                                o_sb[:, _CHUNK + half:], ph[1][:], ident,
                                bias=bias_ap, scale=1.0,
                            )
                            nc.sync.dma_start(
                                yT[s, fh, :,
                                   (c - 1) * _CHUNK:(c + 1) * _CHUNK],
                                o_sb[:],
                            )
                            continue
                        psum = ps.tile([_FH, _CHUNK], f32,
                                       name=f"psum_s{s}f{fh}c{c}", tag="psum")
                        for tap in range(_NTAP):
                            mm = nc.tensor.matmul(
                                psum[:], k_sb[fh][:, tap, :],
                                rhs_ap(x_sb, c, tap),
                                start=(tap == 0), stop=(tap == _NTAP - 1),
                            )
                            if fh == 0 and c == 0 and tap == 0:
                                gate_this = mm.ins
                        # evict with fused bias, alternating engines; DMA out
                        # per pair of chunks
                        if c % 2 == 0:
                            o_sb = op.tile([_FH, 2 * _CHUNK], f16,
                                           name=f"o_s{s}f{fh}p{c // 2}",
                                           tag="o")
                            nc.vector.tensor_scalar_add(
                                o_sb[:, :_CHUNK], psum[:], bias_ap)
                        else:
                            nc.scalar.activation(
                                o_sb[:, _CHUNK:], psum[:], ident,
                                bias=bias_ap, scale=1.0,
                            )
                            nc.sync.dma_start(
                                yT[s, fh, :,
                                   (c - 1) * _CHUNK:(c + 1) * _CHUNK],
                                o_sb[:],
                            )
                gate_prev = gate_this
    nc.compile()
    return nc


def get_nc():
    global _nc
    if _nc is None:
        _nc = _build_nc()
    return _nc


def _prep_inputs(x, classes, kernel, bias):
    cls = np.asarray(classes)[:, 0]
    k_per = np.asarray(kernel)[cls]          # [B, KH, KW, CIN, F]
    b_per = np.asarray(bias)[cls]            # [B, F]

    xpad = np.zeros((_B, _HP, _WP, _CIN), np.float16)
    xpad[:, 1:_H + 1, 1:_W + 1, :] = np.asarray(x).astype(np.float16)
    xT_all = np.ascontiguousarray(xpad.transpose(0, 3, 1, 2))  # [B, CIN, HP, WP]
    # [B, NFH, CIN, NTAP, FH]
    kT_all = np.ascontiguousarray(
        k_per.reshape(_B, _NTAP, _CIN, _NFH, _FH).transpose(0, 3, 2, 1, 4)
    ).astype(np.float16)

    in_maps = []
    for i in range(_NCORES):
        lo = i * _BPC
        b_core = np.ascontiguousarray(
            b_per[lo:lo + _BPC].reshape(_BPC, _NFH, _FH).transpose(2, 0, 1)
        ).reshape(_FH, _BPC * _NFH)
        in_maps.append({
            "xT": np.ascontiguousarray(xT_all[lo:lo + _BPC]),
            "kT": np.ascontiguousarray(kT_all[lo:lo + _BPC]),
            "bT": b_core,
        })
    return in_maps


def _unshard(results):
    outs = []
    for r in results:
        yT = r["yT"].astype(np.float32)       # [BPC, 2, 128, 4096]
        y = yT.reshape(_BPC, _F, _SP).transpose(0, 2, 1)
        outs.append(y.reshape(_BPC, _H, _W, _F))
    return np.ascontiguousarray(np.concatenate(outs, axis=0))


def run(x, classes, kernel, bias, trace=False):
    """Returns (y, BassKernelResults)."""
    from concourse.bass_utils import run_bass_kernel_spmd

    nc = get_nc()
    in_maps = _prep_inputs(x, classes, kernel, bias)
    res = run_bass_kernel_spmd(nc, in_maps, core_ids=list(range(_NCORES)), trace=trace)
    return _unshard(res.results), res


def kernel(x, classes, kernel, bias):
    y, _ = run(x, classes, kernel, bias)
    return y


# revision 8
# speedup vs baseline: 1.0210x; 1.0210x over previous
"""Conditional (class-routed) 3x3 SAME conv, data-parallel over batch on 8 TRN2 cores.

Strategy:
  - Host: gather per-sample expert kernel/bias (kernel[classes], bias[classes]),
    zero-pad x to 66x66 and transpose to channel-major [CIN, HP, WP]; shard
    batch 4 samples/core.
  - Device (per core): for each sample and output-channel half, process the
    64x64 output as 8 spatial chunks of 512 positions, CHUNK-MAJOR: the 9
    conv taps accumulate back-to-back into one PSUM bank ([CIN=128,
    FH=128]^T @ [CIN=128, 512] fp16 matmuls, fp32 PSUM), then the chunk is
    evicted (bias fused) on alternating Vector/Scalar engines while later
    chunks keep the PE busy.
  - x is staged twice in SBUF: plane 0 as-is (kw=0/2 taps) and plane 1
    shifted left one column (kw=1 taps) so every matmul rhs is 4-byte
    aligned (a 2-byte-misaligned rhs costs ~3% per matmul).
  - The first chunk's inputs (k taps 0:3, x rows 0:10) are triggered in
    parallel on the two HWDGE queues (Sync + Scalar) so the PE starts
    ~1us after the first DMA.  The very last chunk accumulates into two
    half-banks so its eviction runs Vector+Scalar in parallel.
  - Host: un-transpose [F, HW] -> [H, W, F] and concatenate shards.
"""

import numpy as np

_B, _H, _W, _CIN = 32, 64, 64, 128
_F, _KH, _KW = 256, 3, 3
_NCORES = 8
_BPC = _B // _NCORES          # 4 samples per core
_HP, _WP = _H + 2, _W + 2     # 66, 66 (zero-padded)
_SP = _H * _W                 # 4096 output positions
_FH = 128                     # output-channel half (PSUM partition dim)
_NFH = _F // _FH              # 2
_CHUNK = 512                  # spatial positions per PSUM bank
_NCH = _SP // _CHUNK          # 8
_ROWS = _CHUNK // _W          # 8 output rows per chunk
_NTAP = _KH * _KW             # 9

_nc = None


def _build_nc():
    import concourse.bacc as bacc
    import concourse.mybir as mybir
    import concourse.tile as tile
    from concourse.tile_rust import add_dep_helper

    f32 = mybir.dt.float32
    f16 = mybir.dt.float16
    ident = mybir.ActivationFunctionType.Identity

    nc = bacc.Bacc("TRN2", target_bir_lowering=False, debug=False)
    xT = nc.dram_tensor("xT", (_BPC, _CIN, _HP, _WP), f16, kind="ExternalInput")
    # x shifted left one column (host-prepared) so kw=1 taps read 4B-aligned
    x2T = nc.dram_tensor("x2T", (_BPC, _CIN, _HP, _WP), f16,
                         kind="ExternalInput")
    # kT[s, fh] is one f-half of the expert kernel: [CIN, NTAP, FH]
    kT = nc.dram_tensor("kT", (_BPC, _NFH, _CIN, _NTAP, _FH), f16,
                        kind="ExternalInput")
    bT = nc.dram_tensor("bT", (_FH, _BPC * _NFH), f32, kind="ExternalInput")
    yT = nc.dram_tensor("yT", (_BPC, _NFH, _FH, _SP), f16, kind="ExternalOutput")

    def rhs_ap(x_sb, c, tap, nrows=_ROWS, roff=0):
        kh, kw = divmod(tap, _KW)
        r0 = c * _ROWS + kh + roff
        if kw == 1:
            return x_sb[1][:, r0:r0 + nrows, 0:_W]
        return x_sb[0][:, r0:r0 + nrows, kw:kw + _W]

    with tile.TileContext(nc) as tc:
        with (
            tc.tile_pool(name="xp", bufs=3) as xp,
            tc.tile_pool(name="xq", bufs=3) as xq,
            tc.tile_pool(name="kp", bufs=5) as kp,
            tc.tile_pool(name="bp", bufs=1) as bp,
            tc.tile_pool(name="op", bufs=6) as op,
            tc.tile_pool(name="ps", bufs=8, space="PSUM") as ps,
        ):
            b_sb = bp.tile([_FH, _BPC * _NFH], f32)

            def load_x(eng, xt, s, r0, r1, plane):
                src = xT if plane == 0 else x2T
                return eng.dma_start(xt[plane][:, r0:r1, :],
                                     src[s, :, r0:r1, :])

            gate_prev = None
            for s in range(_BPC):
                k_sb = []
                dmas = []
                # x plane 0: as-is; plane 1: shifted left one column
                x_sb = (xp.tile([_CIN, _HP, _WP], f16, name=f"x_s{s}", tag="x"),
                        xq.tile([_CIN, _HP, _WP], f16, name=f"x2_s{s}",
                                tag="x2"))
                for fh in range(_NFH):
                    k_sb.append(kp.tile([_CIN, _NTAP, _FH], f16,
                                        name=f"k_s{s}f{fh}", tag="k"))
                if s == 0:
                    # interleave the two HWDGE trigger queues so the first
                    # chunk's inputs land ~1.3us after the first trigger
                    dmas.append(nc.sync.dma_start(k_sb[0][:, 0:3, :],
                                                  kT[s, 0, :, 0:3, :]))
                    dmas.append(load_x(nc.scalar, x_sb, s, 0, 10, 0))
                    dmas.append(load_x(nc.sync, x_sb, s, 0, 10, 1))
                    dmas.append(load_x(nc.scalar, x_sb, s, 10, 18, 0))
                    dmas.append(load_x(nc.sync, x_sb, s, 10, 18, 1))
                    dmas.append(nc.scalar.dma_start(k_sb[0][:, 3:_NTAP, :],
                                                    kT[s, 0, :, 3:_NTAP, :]))
                    dmas.append(load_x(nc.sync, x_sb, s, 18, 42, 0))
                    dmas.append(load_x(nc.scalar, x_sb, s, 18, 42, 1))
                    dmas.append(load_x(nc.sync, x_sb, s, 42, _HP, 0))
                    dmas.append(load_x(nc.scalar, x_sb, s, 42, _HP, 1))
                    dmas.append(nc.sync.dma_start(k_sb[1][:], kT[s, 1]))
                    nc.scalar.dma_start(b_sb[:], bT[:])
                else:
                    dmas.append(nc.sync.dma_start(k_sb[0][:], kT[s, 0]))
                    dmas.append(load_x(nc.scalar, x_sb, s, 0, 34, 0))
                    dmas.append(load_x(nc.sync, x_sb, s, 0, 34, 1))
                    dmas.append(load_x(nc.scalar, x_sb, s, 34, _HP, 0))
                    dmas.append(load_x(nc.sync, x_sb, s, 34, _HP, 1))
                    dmas.append(nc.scalar.dma_start(k_sb[1][:], kT[s, 1]))
                if gate_prev is not None:
                    # prefetch of sample s must not compete for HBM bandwidth
                    # with sample s-1's (still critical) input transfers
                    for d in dmas:
                        add_dep_helper(d.ins, gate_prev,
                                       reason="prefetch gated on prev sample")

                gate_this = None
                for fh in range(_NFH):
                    col = s * _NFH + fh
                    bias_ap = b_sb[:, col:col + 1]
                    o_sb = None
                    for c in range(_NCH):
                        split_last = (s == _BPC - 1 and fh == _NFH - 1
                                      and c == _NCH - 1)
                        if split_last:
                            # two half-banks so both evict engines can run
                            # in parallel on different banks at the very end
                            ph = [ps.tile([_FH, _CHUNK // 2], f32,
                                          name=f"psum_s{s}f{fh}c{c}h{h}",
                                          tag="psum") for h in range(2)]
                            for tap in range(_NTAP):
                                for h in range(2):
                                    nc.tensor.matmul(
                                        ph[h][:],
                                        k_sb[fh][:, tap, :],
                                        rhs_ap(x_sb, c, tap,
                                               nrows=_ROWS // 2,
                                               roff=h * _ROWS // 2),
                                        start=(tap == 0),
                                        stop=(tap == _NTAP - 1),
                                    )
                            half = _CHUNK // 2
                            nc.vector.tensor_scalar_add(
                                o_sb[:, _CHUNK:_CHUNK + half], ph[0][:],
                                bias_ap)
                            nc.scalar.activation(
                                o_sb[:, _CHUNK + half:], ph[1][:], ident,
                                bias=bias_ap, scale=1.0,
                            )
                            nc.sync.dma_start(
                                yT[s, fh, :,
                                   (c - 1) * _CHUNK:(c + 1) * _CHUNK],
                                o_sb[:],
                            )
                            continue
                        psum = ps.tile([_FH, _CHUNK], f32,
                                       name=f"psum_s{s}f{fh}c{c}", tag="psum")
                        for tap in range(_NTAP):
                            mm = nc.tensor.matmul(
                                psum[:], k_sb[fh][:, tap, :],
                                rhs_ap(x_sb, c, tap),
                                start=(tap == 0), stop=(tap == _NTAP - 1),
                            )
                            if fh == 0 and c == 0 and tap == 0:
                                gate_this = mm.ins
                        # evict with fused bias, alternating engines; DMA out
                        # per pair of chunks
                        if c % 2 == 0:
                            o_sb = op.tile([_FH, 2 * _CHUNK], f16,
                                           name=f"o_s{s}f{fh}p{c // 2}",
                                           tag="o")
                            nc.vector.tensor_scalar_add(
                                o_sb[:, :_CHUNK], psum[:], bias_ap)
                        else:
                            nc.scalar.activation(
                                o_sb[:, _CHUNK:], psum[:], ident,
                                bias=bias_ap, scale=1.0,
                            )
                            nc.sync.dma_start(
                                yT[s, fh, :,
                                   (c - 1) * _CHUNK:(c + 1) * _CHUNK],
                                o_sb[:],
                            )
                gate_prev = gate_this
    nc.compile()
    return nc


def get_nc():
    global _nc
    if _nc is None:
        _nc = _build_nc()
    return _nc


def _prep_inputs(x, classes, kernel, bias):
    cls = np.asarray(classes)[:, 0]
    k_per = np.asarray(kernel)[cls]          # [B, KH, KW, CIN, F]
    b_per = np.asarray(bias)[cls]            # [B, F]

    xpad = np.zeros((_B, _HP, _WP + 1, _CIN), np.float16)
    xpad[:, 1:_H + 1, 1:_W + 1, :] = np.asarray(x).astype(np.float16)
    xT_all = np.ascontiguousarray(
        xpad[:, :, :_WP].transpose(0, 3, 1, 2))   # [B, CIN, HP, WP]
    x2T_all = np.ascontiguousarray(
        xpad[:, :, 1:].transpose(0, 3, 1, 2))     # shifted left one column
    # [B, NFH, CIN, NTAP, FH]
    kT_all = np.ascontiguousarray(
        k_per.reshape(_B, _NTAP, _CIN, _NFH, _FH).transpose(0, 3, 2, 1, 4)
    ).astype(np.float16)

    in_maps = []
    for i in range(_NCORES):
        lo = i * _BPC
        b_core = np.ascontiguousarray(
            b_per[lo:lo + _BPC].reshape(_BPC, _NFH, _FH).transpose(2, 0, 1)
        ).reshape(_FH, _BPC * _NFH)
        in_maps.append({
            "xT": np.ascontiguousarray(xT_all[lo:lo + _BPC]),
            "x2T": np.ascontiguousarray(x2T_all[lo:lo + _BPC]),
            "kT": np.ascontiguousarray(kT_all[lo:lo + _BPC]),
            "bT": b_core,
        })
    return in_maps


def _unshard(results):
    outs = []
    for r in results:
        yT = r["yT"].astype(np.float32)       # [BPC, 2, 128, 4096]
        y = yT.reshape(_BPC, _F, _SP).transpose(0, 2, 1)
        outs.append(y.reshape(_BPC, _H, _W, _F))
    return np.ascontiguousarray(np.concatenate(outs, axis=0))


def run(x, classes, kernel, bias, trace=False):
    """Returns (y, BassKernelResults)."""
    from concourse.bass_utils import run_bass_kernel_spmd

    nc = get_nc()
    in_maps = _prep_inputs(x, classes, kernel, bias)
    res = run_bass_kernel_spmd(nc, in_maps, core_ids=list(range(_NCORES)), trace=trace)
    return _unshard(res.results), res


def kernel(x, classes, kernel, bias):
    y, _ = run(x, classes, kernel, bias)
    return y
